# revision 10
# baseline (speedup 1.0000x reference)
"""ALiBi causal attention (B=2, T=2048, C=1024, H=16, D=64, fp32) on 8 trn2 cores.

Sharding: core i -> batch b = i//4, group g = i%4; slot j of core g holds head
4j+g (sorted slope grouping: slot j covers heads {4j..4j+3}, so the compiled
skip pattern for slot j only needs to cover slope(4j+3), the smallest in the
slot, and every core does identical work).

All matmuls in bf16 (1 PE cycle/row at any output width):
  phase 1: xT (C,T) bf16 in SBUF; QT/KT in (d,t) layout with 4 aug rows:
           QTe rows 64..67 = [-slope*t/scale, 1, 1, 1]
           KTe rows 64..67 = [1, h1, h2, h3] where h1+h2+h3 = slope*s/scale
           split across three bf16 rows (24 mantissa bits -> exact through the
           f32 PSUM accumulate; the t-term's bf16 error is constant per t and
           cancels in softmax). V in (t,d) layout + ones column (denominator).
  phase 2: per (head, 512-wide q-block): kept s-tiles from the ALiBi band
           (tau=18) with per-tile column windows; ST pair -> one ACT exp ->
           PT bf16; causal mask via gpsimd affine_select; AV *flipped*:
           avT[t,65] += PT_subtile^T @ V  (65-wide bf16 matmuls, and diagonal
           s-tiles only feed their causal t-subtiles). Normalize per-partition
           (reciprocal + tensor_scalar_mul, no broadcast matmul), then
           DMA-transpose [128,128] tiles into the (d,t) attn layout.
  phase 3: out_partial = attnT^T @ WoT per 128-row strip -> bf16 DMA out.

Emission is software-pipelined (proj chunk k+1 / out-proj strips of qb-1
interleave with attention of qb) and all input DMAs are bf16, split across
the SP and ACT HWDGE queues. Host sums the 4 bf16 partials per batch.
"""

import math
import sys

import numpy as np

for _p in ("/opt/trn_rl_repo", "/root/.axon_site/_ro/trn_rl_repo"):
    try:
        import concourse  # noqa: F401
        break
    except ImportError:
        if _p not in sys.path:
            sys.path.insert(0, _p)

B, T, C, H, D = 2, 2048, 1024, 16, 64
HPC = 4          # heads per core (one per slot)
CS = HPC * D     # 256 channels per core
SCALE = D ** -0.5
NCORES = 8
KAUG = 68        # 64 head dims + t-term row + 3-way s-term split
TAU = 18.0       # drop softmax terms with slope*gap > TAU (< 4e-5 rel mass)

NQT = T // 128   # 16 s/t tiles
NQB = T // 512   # 4 q blocks
KT_C = C // 128  # 8 contraction chunks for projections


def _slopes(n_heads: int) -> np.ndarray:
    i = np.arange(1, n_heads + 1, dtype=np.float64)
    return np.power(2.0, -8.0 * i / n_heads)


# worst (smallest) slope in slot j is head 4j+3
_SLOT_SIG = [float(_slopes(H)[4 * j + 3]) for j in range(HPC)]
_TSIG = [TAU / s for s in _SLOT_SIG]


def _window(j: int, qb: int, st: int):
    """Column window [lo, hi) of s-tile st within q-block qb for slot j,
    or None if the whole tile is below the ALiBi band."""
    rel = st - 4 * qb
    if rel > 3:
        return None
    hi = 128 * rel + 128 + _TSIG[j]
    hi = min(512, 128 * math.ceil(hi / 128))
    lo = max(0, 128 * rel)
    if hi <= lo:
        return None
    return lo, int(hi)


def _kept(j: int, qb: int):
    out = []
    for st in range(4 * qb + 4):
        w = _window(j, qb, st)
        if w is not None:
            out.append((st, w[0], w[1]))
    return out


_PROGRAM = None


def _build_program():
    from contextlib import ExitStack

    import concourse.tile as tile
    from concourse import bacc, mybir

    f32 = mybir.dt.float32
    bf16 = mybir.dt.bfloat16
    EXP = mybir.ActivationFunctionType.Exp

    nc = bacc.Bacc("TRN2", target_bir_lowering=False, debug=False,
                   num_devices=NCORES)
    xT = nc.declare_dram_parameter("xT", [C, T], bf16, isOutput=False)
    wqT = nc.declare_dram_parameter("wqT", [C, CS], bf16, isOutput=False)
    wkT = nc.declare_dram_parameter("wkT", [C, CS], bf16, isOutput=False)
    wvT = nc.declare_dram_parameter("wvT", [C, CS], bf16, isOutput=False)
    woT = nc.declare_dram_parameter("woT", [CS, C], bf16, isOutput=False)
    qaug = nc.declare_dram_parameter("qaug", [4 * HPC, T], bf16, isOutput=False)
    kaug = nc.declare_dram_parameter("kaug", [4 * HPC, T], bf16, isOutput=False)
    out = nc.declare_dram_parameter("out", [T, C], bf16, isOutput=True)

    with nc.allow_low_precision(reason="bf16 compute, f32 PSUM accumulate"), \
         tile.TileContext(nc) as tc, ExitStack() as ctx:
        sb = ctx.enter_context(tc.tile_pool(name="sb", bufs=1))
        psum = ctx.enter_context(tc.tile_pool(name="psum", bufs=1, space="PSUM"))
        pt_pool = ctx.enter_context(tc.tile_pool(name="pt", bufs=16))
        td_pool = ctx.enter_context(tc.tile_pool(name="td", bufs=4))
        dn_pool = ctx.enter_context(tc.tile_pool(name="dn", bufs=4))
        ob_pool = ctx.enter_context(tc.tile_pool(name="ob", bufs=2))

        qt_t = [sb.tile([KAUG, T], bf16, tag=f"qt{h}", name=f"qt{h}")
                for h in range(HPC)]
        kt_t = [sb.tile([KAUG, T], bf16, tag=f"kt{h}", name=f"kt{h}")
                for h in range(HPC)]
        v_t = sb.tile([128, NQT, HPC, 65], bf16)
        attn_t = [sb.tile([128, T], bf16, tag=f"at{p}", name=f"at{p}")
                  for p in range(2)]
        xt_sb = sb.tile([128, KT_C, T], bf16)
        wq_sb = sb.tile([128, KT_C, CS], bf16)
        wk_sb = sb.tile([128, KT_C, CS], bf16)
        wv_sb = sb.tile([128, KT_C, CS], bf16)
        wo_sb = sb.tile([128, 2, C], bf16)

        # ---- front DMAs: SP carries the big loads, ACT the small ones.
        # Order matters: the DMA device is a single serial resource, and the
        # first projection matmul needs wq[k0:4] + xt0[k0:4] only.
        xt_view = xT.rearrange("(k p) t -> p k t", p=128)
        wq_view = wqT.rearrange("(k p) c -> p k c", p=128)
        nc.sync.dma_start(out=wq_sb[:, 0:4, :], in_=wq_view[:, 0:4, :])
        nc.sync.dma_start(out=xt_sb[:, 0:4, 0:512], in_=xt_view[:, 0:4, 0:512])
        nc.sync.dma_start(out=wq_sb[:, 4:8, :], in_=wq_view[:, 4:8, :])
        nc.sync.dma_start(out=xt_sb[:, 4:8, 0:512], in_=xt_view[:, 4:8, 0:512])
        nc.sync.dma_start(out=wk_sb[:],
                          in_=wkT.rearrange("(k p) c -> p k c", p=128))
        nc.gpsimd.memset(v_t[:], 1.0)  # ones column preset (Pool is idle)
        for h in range(HPC):
            nc.scalar.dma_start(out=qt_t[h][64:68, :],
                                in_=qaug[4 * h:4 * h + 4, :])
            nc.scalar.dma_start(out=kt_t[h][64:68, :],
                                in_=kaug[4 * h:4 * h + 4, :])
        nc.scalar.dma_start(
            out=wo_sb[:], in_=woT.rearrange("(k p) c -> p k c", p=128))
        nc.sync.dma_start(out=wv_sb[:],
                          in_=wvT.rearrange("(k p) c -> p k c", p=128))
        for tch in range(1, NQB):
            nc.sync.dma_start(out=xt_sb[:, :, tch * 512:(tch + 1) * 512],
                              in_=xt_view[:, :, tch * 512:(tch + 1) * 512])



        cp_rr = [0]

        def copy_rr(dst, src):
            """Round-robin PSUM->SBUF copies between DVE and ACT (gpsimd
            cannot touch PSUM on real hardware)."""
            cp_rr[0] += 1
            if cp_rr[0] % 2 == 0:
                nc.vector.tensor_copy(dst, src)
            else:
                nc.scalar.copy(dst, src)

        def emit_proj_qk(tch):
            for (w_sb, dst) in ((wq_sb, qt_t), (wk_sb, kt_t)):
                for dt_i in range(2):
                    ps = psum.tile([128, 512], f32, tag="p1", bufs=2,
                                   name="qkps")
                    for k in range(KT_C):
                        nc.tensor.matmul(
                            ps[:],
                            lhsT=w_sb[:, k, dt_i * 128:(dt_i + 1) * 128],
                            rhs=xt_sb[:, k, tch * 512:(tch + 1) * 512],
                            start=(k == 0), stop=(k == KT_C - 1),
                        )
                    for hl in range(2):
                        h = dt_i * 2 + hl
                        copy_rr(
                            dst[h][0:64, tch * 512:(tch + 1) * 512],
                            ps[hl * 64:(hl + 1) * 64, :])

        def emit_proj_v(tch):
            for st in range(4 * tch, 4 * tch + 4):
                ps = psum.tile([128, 512], f32, tag="p1", bufs=2,
                               name=f"vps{st}")
                for k in range(KT_C):
                    nc.tensor.matmul(
                        ps[:, 0:CS],
                        lhsT=xt_sb[:, k, st * 128:(st + 1) * 128],
                        rhs=wv_sb[:, k, :],
                        start=(k == 0), stop=(k == KT_C - 1),
                    )
                nc.vector.tensor_copy(
                    v_t[:, st, :, 0:64],
                    ps[:, 0:CS].rearrange("p (h d) -> p h d", h=HPC))

        # attn_td tiles per (qb, pair): written by norm-muls, read by the
        # DMA transpose into attn_t
        td_tiles = {}

        def emit_att_st(qb, h):
            """ST matmuls + exp + causal mask for group (qb, h); the AV stage
            is deferred one group (psum accumulation groups are bank-scoped,
            so the four t-subtile accumulations must run jj-outer, which
            needs every pt of the group alive)."""
            kept = _kept(h, qb)
            pairs = [kept[i:i + 2] for i in range(0, len(kept), 2)]
            pt_of = {}
            for pr in pairs:
                sp = psum.tile([128, 2, 512], f32, tag="sp", bufs=2, name="sp")
                # both tiles compute the union window so one exp covers the
                # pair; extra columns are future-masked or below e^-TAU
                ulo = min(w[1] for w in pr)
                uhi = max(w[2] for w in pr)
                for j, (st, lo, hi) in enumerate(pr):
                    nc.tensor.matmul(
                        sp[:, j, ulo:uhi],
                        lhsT=kt_t[h][:, st * 128:(st + 1) * 128],
                        rhs=qt_t[h][:, qb * 512 + ulo:qb * 512 + uhi],
                        start=True, stop=True,
                    )
                pt = pt_pool.tile([128, 2, 512], bf16, tag="pt", name="ptt")
                npair = len(pr)
                nc.scalar.activation(pt[:, 0:npair, ulo:uhi],
                                     sp[:, 0:npair, ulo:uhi], EXP, scale=SCALE)
                for j, (st, lo, hi) in enumerate(pr):
                    rel = st - 4 * qb
                    if rel >= 0:  # causal mask on diagonal tiles
                        nc.gpsimd.affine_select(
                            pt[:, j, lo:hi], pt[:, j, lo:hi],
                            pattern=[[1, hi - lo]],
                            compare_op=mybir.AluOpType.is_ge,
                            fill=0.0, base=0,
                            channel_multiplier=-1,
                        )
                    pt_of[st] = (pt, j, hi)
            return (qb, h, kept, pt_of)

        def emit_att_av(ctx):
            """AV accumulation (jj-outer), normalize, transpose to (d,t)."""
            qb, h, kept, pt_of = ctx
            av_sts = [[st for (st, lo, hi) in kept
                       if hi >= (jj + 1) * 128 and lo <= jj * 128]
                      for jj in range(4)]
            av = psum.tile([128, 4, 65], f32, tag="av", bufs=2, name="av")
            for jj in range(4):
                sts = av_sts[jj]
                for st in sts:
                    pt, j, _ = pt_of[st]
                    nc.tensor.matmul(
                        av[:, jj, :],
                        lhsT=pt[:, j, jj * 128:(jj + 1) * 128],
                        rhs=v_t[:, st, h, :],
                        start=(st == sts[0]), stop=(st == sts[-1]),
                    )
            # normalize: per-partition (q position) reciprocal multiply
            p = h // 2
            key = (qb, p)
            if key not in td_tiles:
                td_tiles[key] = td_pool.tile([128, 4, 2, 64], bf16, tag="td",
                                             name=f"td{qb}_{p}")
            td = td_tiles[key]
            den = dn_pool.tile([128, 4], f32, tag="den", name="den")
            nc.vector.reciprocal(den[:], av[:, :, 64])
            for jj in range(4):
                nc.vector.tensor_scalar_mul(
                    td[:, jj, h % 2, :], av[:, jj, 0:64], den[:, jj:jj + 1])
            if h % 2 == 0:  # second head of the pair: transpose to (d, t)
                for jj in range(4):
                    nc.sync.dma_start_transpose(
                        out=attn_t[p][:, qb * 512 + jj * 128:
                                      qb * 512 + (jj + 1) * 128],
                        in_=td[:, jj, :, :])
                del td_tiles[key]

        def emit_outproj(qb, strips):
            for qt_i in [4 * qb + s for s in strips]:
                ob = ob_pool.tile([128, C], bf16, tag="ob", name="ob")
                for chn in range(2):
                    ps = psum.tile([128, 512], f32, tag="p1", bufs=2,
                                   name="ops")
                    for i, kt_i in enumerate((1, 0)):
                        nc.tensor.matmul(
                            ps[:],
                            lhsT=attn_t[kt_i][:, qt_i * 128:(qt_i + 1) * 128],
                            rhs=wo_sb[:, kt_i, chn * 512:(chn + 1) * 512],
                            start=(i == 0), stop=(i == 1),
                        )
                    copy_rr(ob[:, chn * 512:(chn + 1) * 512], ps[:])
                nc.sync.dma_start(
                    out=out[qt_i * 128:(qt_i + 1) * 128, :], in_=ob[:])

        # ---- software-pipelined emission: the AV stage of each attention
        # group is deferred one group so PE never waits on that group's exps
        emit_proj_qk(0)
        emit_proj_v(0)
        pend = None

        def att(qb, h):
            nonlocal pend
            nxt = emit_att_st(qb, h)
            if pend is not None:
                emit_att_av(pend)
            pend = nxt

        for h in (3, 2, 1, 0):
            att(0, h)
        for qb in range(1, NQB):
            emit_proj_qk(qb)
            emit_proj_v(qb)
            att(qb, 3)
            att(qb, 2)
            emit_outproj(qb - 1, (0, 1))
            att(qb, 1)
            att(qb, 0)
            emit_outproj(qb - 1, (2, 3))
        emit_att_av(pend)
        emit_outproj(NQB - 1, (0, 1, 2, 3))

    nc.finalize()
    return nc


def _bf16(a):
    import ml_dtypes
    return np.asarray(a, np.float32).astype(ml_dtypes.bfloat16)


def _host_inputs(x, Wq, Wk, Wv, Wo):
    import ml_dtypes
    bf = ml_dtypes.bfloat16
    slopes = _slopes(H)
    t_idx = np.arange(T, dtype=np.float64)
    in_maps = []
    for core in range(NCORES):
        b, g = core // 4, core % 4
        heads = [4 * j + g for j in range(HPC)]
        hs = np.concatenate([np.arange(h * D, (h + 1) * D) for h in heads])
        qa = np.empty((4 * HPC, T), dtype=bf)
        ka = np.empty((4 * HPC, T), dtype=bf)
        for j in range(HPC):
            sig = slopes[heads[j]]
            v = sig * t_idx / SCALE
            h1 = v.astype(bf)
            h2 = (v - h1.astype(np.float64)).astype(bf)
            h3 = (v - h1.astype(np.float64) - h2.astype(np.float64)).astype(bf)
            qa[4 * j] = (-v).astype(bf)
            qa[4 * j + 1:4 * j + 4] = bf(1.0)
            ka[4 * j] = bf(1.0)
            ka[4 * j + 1] = h1
            ka[4 * j + 2] = h2
            ka[4 * j + 3] = h3
        in_maps.append({
            "xT": _bf16(np.ascontiguousarray(x[b].T)),
            "wqT": _bf16(np.ascontiguousarray(Wq[hs, :].T)),
            "wkT": _bf16(np.ascontiguousarray(Wk[hs, :].T)),
            "wvT": _bf16(np.ascontiguousarray(Wv[hs, :].T)),
            "woT": _bf16(np.ascontiguousarray(Wo[:, hs].T)),
            "qaug": qa,
            "kaug": ka,
        })
    return in_maps


def get_program():
    global _PROGRAM
    if _PROGRAM is None:
        _PROGRAM = _build_program()
    return _PROGRAM


def kernel(x, Wq, Wk, Wv, Wo, _trace=False):
    from concourse.bass_utils import run_bass_kernel_spmd

    x = np.asarray(x, dtype=np.float32)
    nc = get_program()
    in_maps = _host_inputs(x, np.asarray(Wq, np.float32),
                           np.asarray(Wk, np.float32),
                           np.asarray(Wv, np.float32),
                           np.asarray(Wo, np.float32))
    res = run_bass_kernel_spmd(nc, in_maps, list(range(NCORES)), trace=_trace)
    kernel.last_results = res
    outs = [np.asarray(res.results[i]["out"], dtype=np.float32)
            for i in range(NCORES)]
    full = np.empty((B, T, C), dtype=np.float32)
    for b in range(B):
        full[b] = outs[4 * b] + outs[4 * b + 1] + outs[4 * b + 2] + outs[4 * b + 3]
    return full


# revision 11
# speedup vs baseline: 1.0070x; 1.0070x over previous
"""ALiBi causal attention (B=2, T=2048, C=1024, H=16, D=64, fp32) on 8 trn2 cores.

Sharding: core i -> batch b = i//4, group g = i%4; slot j of core g holds head
4j+g (sorted slope grouping: slot j covers heads {4j..4j+3}, so the compiled
skip pattern for slot j only needs to cover slope(4j+3), the smallest in the
slot, and every core does identical work).

All matmuls in bf16 (1 PE cycle/row at any output width):
  phase 1: xT (C,T) bf16 in SBUF; QT/KT in (d,t) layout with 4 aug rows:
           QTe rows 64..67 = [-slope*t/scale, 1, 1, 1]
           KTe rows 64..67 = [1, h1, h2, h3] where h1+h2+h3 = slope*s/scale
           split across three bf16 rows (24 mantissa bits -> exact through the
           f32 PSUM accumulate; the t-term's bf16 error is constant per t and
           cancels in softmax). V in (t,d) layout + ones column (denominator).
  phase 2: per (head, 512-wide q-block): kept s-tiles from the ALiBi band
           (tau=18) with per-tile column windows; ST pair -> one ACT exp ->
           PT bf16; causal mask via gpsimd affine_select; AV *flipped*:
           avT[t,65] += PT_subtile^T @ V  (65-wide bf16 matmuls, and diagonal
           s-tiles only feed their causal t-subtiles). Normalize per-partition
           (reciprocal + tensor_scalar_mul, no broadcast matmul), then
           DMA-transpose [128,128] tiles into the (d,t) attn layout.
  phase 3: out_partial = attnT^T @ WoT per 128-row strip -> bf16 DMA out.

Emission is software-pipelined (proj chunk k+1 / out-proj strips of qb-1
interleave with attention of qb) and all input DMAs are bf16, split across
the SP and ACT HWDGE queues. Host sums the 4 bf16 partials per batch.
"""

import math
import sys

import numpy as np

for _p in ("/opt/trn_rl_repo", "/root/.axon_site/_ro/trn_rl_repo"):
    try:
        import concourse  # noqa: F401
        break
    except ImportError:
        if _p not in sys.path:
            sys.path.insert(0, _p)

B, T, C, H, D = 2, 2048, 1024, 16, 64
HPC = 4          # heads per core (one per slot)
CS = HPC * D     # 256 channels per core
SCALE = D ** -0.5
NCORES = 8
KAUG = 68        # 64 head dims + t-term row + 3-way s-term split
TAU = 18.0       # drop softmax terms with slope*gap > TAU (< 4e-5 rel mass)

NQT = T // 128   # 16 s/t tiles
NQB = T // 512   # 4 q blocks
KT_C = C // 128  # 8 contraction chunks for projections


def _slopes(n_heads: int) -> np.ndarray:
    i = np.arange(1, n_heads + 1, dtype=np.float64)
    return np.power(2.0, -8.0 * i / n_heads)


# worst (smallest) slope in slot j is head 4j+3
_SLOT_SIG = [float(_slopes(H)[4 * j + 3]) for j in range(HPC)]
_TSIG = [TAU / s for s in _SLOT_SIG]


def _window(j: int, qb: int, st: int):
    """Column window [lo, hi) of s-tile st within q-block qb for slot j,
    or None if the whole tile is below the ALiBi band."""
    rel = st - 4 * qb
    if rel > 3:
        return None
    hi = 128 * rel + 128 + _TSIG[j]
    hi = min(512, 128 * math.ceil(hi / 128))
    lo = max(0, 128 * rel)
    if hi <= lo:
        return None
    return lo, int(hi)


def _kept(j: int, qb: int):
    out = []
    for st in range(4 * qb + 4):
        w = _window(j, qb, st)
        if w is not None:
            out.append((st, w[0], w[1]))
    return out


_PROGRAM = None


def _build_program():
    from contextlib import ExitStack

    import concourse.tile as tile
    from concourse import bacc, mybir

    f32 = mybir.dt.float32
    bf16 = mybir.dt.bfloat16
    EXP = mybir.ActivationFunctionType.Exp

    nc = bacc.Bacc("TRN2", target_bir_lowering=False, debug=False,
                   num_devices=NCORES)
    xT = nc.declare_dram_parameter("xT", [C, T], bf16, isOutput=False)
    wqT = nc.declare_dram_parameter("wqT", [C, CS], bf16, isOutput=False)
    wkT = nc.declare_dram_parameter("wkT", [C, CS], bf16, isOutput=False)
    wvT = nc.declare_dram_parameter("wvT", [C, CS], bf16, isOutput=False)
    woT = nc.declare_dram_parameter("woT", [CS, C], bf16, isOutput=False)
    qaug = nc.declare_dram_parameter("qaug", [4 * HPC, T], bf16, isOutput=False)
    kaug = nc.declare_dram_parameter("kaug", [4 * HPC, T], bf16, isOutput=False)
    out = nc.declare_dram_parameter("out", [T, C], bf16, isOutput=True)

    with nc.allow_low_precision(reason="bf16 compute, f32 PSUM accumulate"), \
         tile.TileContext(nc) as tc, ExitStack() as ctx:
        sb = ctx.enter_context(tc.tile_pool(name="sb", bufs=1))
        psum = ctx.enter_context(tc.tile_pool(name="psum", bufs=1, space="PSUM"))
        pt_pool = ctx.enter_context(tc.tile_pool(name="pt", bufs=16))
        td_pool = ctx.enter_context(tc.tile_pool(name="td", bufs=4))
        dn_pool = ctx.enter_context(tc.tile_pool(name="dn", bufs=4))
        ob_pool = ctx.enter_context(tc.tile_pool(name="ob", bufs=2))

        qt_t = [sb.tile([KAUG, T], bf16, tag=f"qt{h}", name=f"qt{h}")
                for h in range(HPC)]
        kt_t = [sb.tile([KAUG, T], bf16, tag=f"kt{h}", name=f"kt{h}")
                for h in range(HPC)]
        v_t = sb.tile([128, NQT, HPC, 65], bf16)
        attn_t = [sb.tile([128, T], bf16, tag=f"at{p}", name=f"at{p}")
                  for p in range(2)]
        xt_sb = sb.tile([128, KT_C, T], bf16)
        wq_sb = sb.tile([128, KT_C, CS], bf16)
        wk_sb = sb.tile([128, KT_C, CS], bf16)
        wv_sb = sb.tile([128, KT_C, CS], bf16)
        wo_sb = sb.tile([128, 2, C], bf16)

        # ---- front DMAs: SP carries the big loads, ACT the small ones.
        # Order matters: the DMA device is a single serial resource, and the
        # first projection matmul needs wq[k0:4] + xt0[k0:4] only.
        xt_view = xT.rearrange("(k p) t -> p k t", p=128)
        wq_view = wqT.rearrange("(k p) c -> p k c", p=128)
        nc.sync.dma_start(out=wq_sb[:, 0:4, :], in_=wq_view[:, 0:4, :])
        nc.sync.dma_start(out=xt_sb[:, 0:4, 0:512], in_=xt_view[:, 0:4, 0:512])
        nc.sync.dma_start(out=wq_sb[:, 4:8, :], in_=wq_view[:, 4:8, :])
        nc.sync.dma_start(out=xt_sb[:, 4:8, 0:512], in_=xt_view[:, 4:8, 0:512])
        nc.sync.dma_start(out=wk_sb[:],
                          in_=wkT.rearrange("(k p) c -> p k c", p=128))
        nc.gpsimd.memset(v_t[:], 1.0)  # ones column preset (Pool is idle)
        for h in range(HPC):
            nc.sync.dma_start(out=qt_t[h][64:68, :],
                              in_=qaug[4 * h:4 * h + 4, :])
            nc.sync.dma_start(out=kt_t[h][64:68, :],
                              in_=kaug[4 * h:4 * h + 4, :])
        nc.sync.dma_start(out=wv_sb[:],
                          in_=wvT.rearrange("(k p) c -> p k c", p=128))
        for tch in range(1, NQB):
            nc.sync.dma_start(out=xt_sb[:, :, tch * 512:(tch + 1) * 512],
                              in_=xt_view[:, :, tch * 512:(tch + 1) * 512])
        nc.sync.dma_start(
            out=wo_sb[:], in_=woT.rearrange("(k p) c -> p k c", p=128))



        cp_rr = [0]

        def copy_rr(dst, src):
            """Round-robin PSUM->SBUF copies between DVE and ACT (gpsimd
            cannot touch PSUM on real hardware)."""
            cp_rr[0] += 1
            if cp_rr[0] % 2 == 0:
                nc.vector.tensor_copy(dst, src)
            else:
                nc.scalar.copy(dst, src)

        def emit_proj_qk(tch):
            for (w_sb, dst) in ((wq_sb, qt_t), (wk_sb, kt_t)):
                for dt_i in range(2):
                    ps = psum.tile([128, 512], f32, tag="p1", bufs=2,
                                   name="qkps")
                    for k in range(KT_C):
                        nc.tensor.matmul(
                            ps[:],
                            lhsT=w_sb[:, k, dt_i * 128:(dt_i + 1) * 128],
                            rhs=xt_sb[:, k, tch * 512:(tch + 1) * 512],
                            start=(k == 0), stop=(k == KT_C - 1),
                        )
                    for hl in range(2):
                        h = dt_i * 2 + hl
                        copy_rr(
                            dst[h][0:64, tch * 512:(tch + 1) * 512],
                            ps[hl * 64:(hl + 1) * 64, :])

        def emit_proj_v(tch):
            for st in range(4 * tch, 4 * tch + 4):
                ps = psum.tile([128, 512], f32, tag="p1", bufs=2,
                               name=f"vps{st}")
                for k in range(KT_C):
                    nc.tensor.matmul(
                        ps[:, 0:CS],
                        lhsT=xt_sb[:, k, st * 128:(st + 1) * 128],
                        rhs=wv_sb[:, k, :],
                        start=(k == 0), stop=(k == KT_C - 1),
                    )
                nc.vector.tensor_copy(
                    v_t[:, st, :, 0:64],
                    ps[:, 0:CS].rearrange("p (h d) -> p h d", h=HPC))

        # attn_td tiles per (qb, pair): written by norm-muls, read by the
        # DMA transpose into attn_t
        td_tiles = {}

        def emit_att_st(qb, h):
            """ST matmuls + exp + causal mask for group (qb, h); the AV stage
            is deferred one group (psum accumulation groups are bank-scoped,
            so the four t-subtile accumulations must run jj-outer, which
            needs every pt of the group alive)."""
            kept = _kept(h, qb)
            pairs = [kept[i:i + 2] for i in range(0, len(kept), 2)]
            pt_of = {}
            for pr in pairs:
                sp = psum.tile([128, 2, 512], f32, tag="sp", bufs=2, name="sp")
                # both tiles compute the union window so one exp covers the
                # pair; extra columns are future-masked or below e^-TAU
                ulo = min(w[1] for w in pr)
                uhi = max(w[2] for w in pr)
                for j, (st, lo, hi) in enumerate(pr):
                    nc.tensor.matmul(
                        sp[:, j, ulo:uhi],
                        lhsT=kt_t[h][:, st * 128:(st + 1) * 128],
                        rhs=qt_t[h][:, qb * 512 + ulo:qb * 512 + uhi],
                        start=True, stop=True,
                    )
                pt = pt_pool.tile([128, 2, 512], bf16, tag="pt", name="ptt")
                npair = len(pr)
                nc.scalar.activation(pt[:, 0:npair, ulo:uhi],
                                     sp[:, 0:npair, ulo:uhi], EXP, scale=SCALE)
                for j, (st, lo, hi) in enumerate(pr):
                    rel = st - 4 * qb
                    if rel >= 0:  # causal mask on diagonal tiles
                        nc.gpsimd.affine_select(
                            pt[:, j, lo:hi], pt[:, j, lo:hi],
                            pattern=[[1, hi - lo]],
                            compare_op=mybir.AluOpType.is_ge,
                            fill=0.0, base=0,
                            channel_multiplier=-1,
                        )
                    pt_of[st] = (pt, j, hi)
            return (qb, h, kept, pt_of)

        def emit_att_av(ctx):
            """AV accumulation (jj-outer), normalize, transpose to (d,t)."""
            qb, h, kept, pt_of = ctx
            av_sts = [[st for (st, lo, hi) in kept
                       if hi >= (jj + 1) * 128 and lo <= jj * 128]
                      for jj in range(4)]
            av = psum.tile([128, 4, 65], f32, tag="av", bufs=2, name="av")
            for jj in range(4):
                sts = av_sts[jj]
                for st in sts:
                    pt, j, _ = pt_of[st]
                    nc.tensor.matmul(
                        av[:, jj, :],
                        lhsT=pt[:, j, jj * 128:(jj + 1) * 128],
                        rhs=v_t[:, st, h, :],
                        start=(st == sts[0]), stop=(st == sts[-1]),
                    )
            # normalize: per-partition (q position) reciprocal multiply
            p = h // 2
            key = (qb, p)
            if key not in td_tiles:
                td_tiles[key] = td_pool.tile([128, 4, 2, 64], bf16, tag="td",
                                             name=f"td{qb}_{p}")
            td = td_tiles[key]
            den = dn_pool.tile([128, 4], f32, tag="den", name="den")
            nc.vector.reciprocal(den[:], av[:, :, 64])
            for jj in range(4):
                nc.vector.tensor_scalar_mul(
                    td[:, jj, h % 2, :], av[:, jj, 0:64], den[:, jj:jj + 1])
            if h % 2 == 0:  # second head of the pair: transpose to (d, t)
                for jj in range(4):
                    nc.sync.dma_start_transpose(
                        out=attn_t[p][:, qb * 512 + jj * 128:
                                      qb * 512 + (jj + 1) * 128],
                        in_=td[:, jj, :, :])
                del td_tiles[key]

        def emit_outproj(qb, strips):
            for qt_i in [4 * qb + s for s in strips]:
                ob = ob_pool.tile([128, C], bf16, tag="ob", name="ob")
                for chn in range(2):
                    ps = psum.tile([128, 512], f32, tag="p1", bufs=2,
                                   name="ops")
                    for i, kt_i in enumerate((1, 0)):
                        nc.tensor.matmul(
                            ps[:],
                            lhsT=attn_t[kt_i][:, qt_i * 128:(qt_i + 1) * 128],
                            rhs=wo_sb[:, kt_i, chn * 512:(chn + 1) * 512],
                            start=(i == 0), stop=(i == 1),
                        )
                    copy_rr(ob[:, chn * 512:(chn + 1) * 512], ps[:])
                nc.sync.dma_start(
                    out=out[qt_i * 128:(qt_i + 1) * 128, :], in_=ob[:])

        # ---- software-pipelined emission: the AV stage of each attention
        # group is deferred one group so PE never waits on that group's exps
        emit_proj_qk(0)
        emit_proj_v(0)
        pend = None

        def att(qb, h):
            nonlocal pend
            nxt = emit_att_st(qb, h)
            if pend is not None:
                emit_att_av(pend)
            pend = nxt

        for h in (3, 2, 1, 0):
            att(0, h)
        for qb in range(1, NQB):
            emit_proj_qk(qb)
            emit_proj_v(qb)
            att(qb, 3)
            att(qb, 2)
            emit_outproj(qb - 1, (0, 1))
            att(qb, 1)
            att(qb, 0)
            emit_outproj(qb - 1, (2, 3))
        emit_att_av(pend)
        emit_outproj(NQB - 1, (0, 1, 2, 3))

    nc.finalize()
    return nc


def _bf16(a):
    import ml_dtypes
    return np.asarray(a, np.float32).astype(ml_dtypes.bfloat16)


def _host_inputs(x, Wq, Wk, Wv, Wo):
    import ml_dtypes
    bf = ml_dtypes.bfloat16
    slopes = _slopes(H)
    t_idx = np.arange(T, dtype=np.float64)
    in_maps = []
    for core in range(NCORES):
        b, g = core // 4, core % 4
        heads = [4 * j + g for j in range(HPC)]
        hs = np.concatenate([np.arange(h * D, (h + 1) * D) for h in heads])
        qa = np.empty((4 * HPC, T), dtype=bf)
        ka = np.empty((4 * HPC, T), dtype=bf)
        for j in range(HPC):
            sig = slopes[heads[j]]
            v = sig * t_idx / SCALE
            h1 = v.astype(bf)
            h2 = (v - h1.astype(np.float64)).astype(bf)
            h3 = (v - h1.astype(np.float64) - h2.astype(np.float64)).astype(bf)
            qa[4 * j] = (-v).astype(bf)
            qa[4 * j + 1:4 * j + 4] = bf(1.0)
            ka[4 * j] = bf(1.0)
            ka[4 * j + 1] = h1
            ka[4 * j + 2] = h2
            ka[4 * j + 3] = h3
        in_maps.append({
            "xT": _bf16(np.ascontiguousarray(x[b].T)),
            "wqT": _bf16(np.ascontiguousarray(Wq[hs, :].T)),
            "wkT": _bf16(np.ascontiguousarray(Wk[hs, :].T)),
            "wvT": _bf16(np.ascontiguousarray(Wv[hs, :].T)),
            "woT": _bf16(np.ascontiguousarray(Wo[:, hs].T)),
            "qaug": qa,
            "kaug": ka,
        })
    return in_maps


def get_program():
    global _PROGRAM
    if _PROGRAM is None:
        _PROGRAM = _build_program()
    return _PROGRAM


def kernel(x, Wq, Wk, Wv, Wo, _trace=False):
    from concourse.bass_utils import run_bass_kernel_spmd

    x = np.asarray(x, dtype=np.float32)
    nc = get_program()
    in_maps = _host_inputs(x, np.asarray(Wq, np.float32),
                           np.asarray(Wk, np.float32),
                           np.asarray(Wv, np.float32),
                           np.asarray(Wo, np.float32))
    res = run_bass_kernel_spmd(nc, in_maps, list(range(NCORES)), trace=_trace)
    kernel.last_results = res
    outs = [np.asarray(res.results[i]["out"], dtype=np.float32)
            for i in range(NCORES)]
    full = np.empty((B, T, C), dtype=np.float32)
    for b in range(B):
        full[b] = outs[4 * b] + outs[4 * b + 1] + outs[4 * b + 2] + outs[4 * b + 3]
    return full


# revision 23
# speedup vs baseline: 1.0723x; 1.0648x over previous
"""ALiBi causal attention (B=2, T=2048, C=1024, H=16, D=64, fp32) on 8 trn2 cores.

Sharding: core i -> batch b = i//4, group g = i%4; slot j of core g holds head
4j+g (sorted slope grouping: slot j covers heads {4j..4j+3}, so the compiled
skip pattern for slot j only needs to cover slope(4j+3), the smallest in the
slot, and every core does identical work).

All matmuls in bf16 (1 PE cycle/row at any output width):
  phase 1: xT (C,T) bf16 in SBUF; QT/KT in (d,t) layout with 4 aug rows:
           QTe rows 64..67 = [-slope*t/scale, 1, 1, 1]
           KTe rows 64..67 = [1, h1, h2, h3] where h1+h2+h3 = slope*s/scale
           split across three bf16 rows (24 mantissa bits -> exact through the
           f32 PSUM accumulate; the t-term's bf16 error is constant per t and
           cancels in softmax). V in (t,d) layout + ones column (denominator).
  phase 2: per (head, 512-wide q-block): kept s-tiles from the ALiBi band
           (tau=18) with per-tile column windows; ST pair -> one ACT exp ->
           PT bf16; causal mask via gpsimd affine_select; AV *flipped*:
           avT[t,65] += PT_subtile^T @ V  (65-wide bf16 matmuls, and diagonal
           s-tiles only feed their causal t-subtiles). Normalize per-partition
           (reciprocal + tensor_scalar_mul, no broadcast matmul), then
           DMA-transpose [128,128] tiles into the (d,t) attn layout.
  phase 3: out_partial = attnT^T @ WoT per 128-row strip -> bf16 DMA out.

Emission is software-pipelined (proj chunk k+1 / out-proj strips of qb-1
interleave with attention of qb) and all input DMAs are bf16, split across
the SP and ACT HWDGE queues. Host sums the 4 bf16 partials per batch.
"""

import math
import sys

import numpy as np

for _p in ("/opt/trn_rl_repo", "/root/.axon_site/_ro/trn_rl_repo"):
    try:
        import concourse  # noqa: F401
        break
    except ImportError:
        if _p not in sys.path:
            sys.path.insert(0, _p)

B, T, C, H, D = 2, 2048, 1024, 16, 64
HPC = 4          # heads per core (one per slot)
CS = HPC * D     # 256 channels per core
SCALE = D ** -0.5
NCORES = 8
KAUG = 68        # 64 head dims + t-term row + 3-way s-term split
TAU = 18.0       # drop softmax terms with slope*gap > TAU (< 4e-5 rel mass)

NQT = T // 128   # 16 s/t tiles
NQB = T // 512   # 4 q blocks
KT_C = C // 128  # 8 contraction chunks for projections


def _slopes(n_heads: int) -> np.ndarray:
    i = np.arange(1, n_heads + 1, dtype=np.float64)
    return np.power(2.0, -8.0 * i / n_heads)


# worst (smallest) slope in slot j is head 4j+3
_SLOT_SIG = [float(_slopes(H)[4 * j + 3]) for j in range(HPC)]
_TSIG = [TAU / s for s in _SLOT_SIG]


def _window(j: int, qb: int, st: int):
    """Column window [lo, hi) of s-tile st within q-block qb for slot j,
    or None if the whole tile is below the ALiBi band."""
    rel = st - 4 * qb
    if rel > 3:
        return None
    hi = 128 * rel + 128 + _TSIG[j]
    hi = min(512, 128 * math.ceil(hi / 128))
    lo = max(0, 128 * rel)
    if hi <= lo:
        return None
    return lo, int(hi)


def _kept(j: int, qb: int):
    out = []
    for st in range(4 * qb + 4):
        w = _window(j, qb, st)
        if w is not None:
            out.append((st, w[0], w[1]))
    return out


_PROGRAM = None


def _build_program():
    from contextlib import ExitStack

    import concourse.tile as tile
    from concourse import bacc, mybir

    f32 = mybir.dt.float32
    bf16 = mybir.dt.bfloat16
    EXP = mybir.ActivationFunctionType.Exp

    nc = bacc.Bacc("TRN2", target_bir_lowering=False, debug=False,
                   num_devices=NCORES)
    xT = nc.declare_dram_parameter("xT", [C, T], bf16, isOutput=False)
    wqT = nc.declare_dram_parameter("wqT", [C, CS], bf16, isOutput=False)
    wkT = nc.declare_dram_parameter("wkT", [C, CS], bf16, isOutput=False)
    wvT = nc.declare_dram_parameter("wvT", [C, CS], bf16, isOutput=False)
    woT = nc.declare_dram_parameter("woT", [CS, C], bf16, isOutput=False)
    # aug rows padded to [128, T] with head h's 4 rows at partition 32h:
    # DMA cost is per-partition line bytes (partition count is free), and
    # the on-device fan-out copies need start partitions in {0,32,64,96}
    augq = nc.declare_dram_parameter("augq", [128, T], bf16, isOutput=False)
    augk = nc.declare_dram_parameter("augk", [128, T], bf16, isOutput=False)
    out = nc.declare_dram_parameter("out", [T, C], bf16, isOutput=True)

    with nc.allow_low_precision(reason="bf16 compute, f32 PSUM accumulate"), \
         tile.TileContext(nc) as tc, ExitStack() as ctx:
        sb = ctx.enter_context(tc.tile_pool(name="sb", bufs=1))
        psum = ctx.enter_context(tc.tile_pool(name="psum", bufs=1, space="PSUM"))
        pt_pool = ctx.enter_context(tc.tile_pool(name="pt", bufs=24))
        td_pool = ctx.enter_context(tc.tile_pool(name="td", bufs=4))
        dn_pool = ctx.enter_context(tc.tile_pool(name="dn", bufs=4))
        ob_pool = ctx.enter_context(tc.tile_pool(name="ob", bufs=2))

        qt_t = [sb.tile([KAUG, T], bf16, tag=f"qt{h}", name=f"qt{h}")
                for h in range(HPC)]
        kt_t = [sb.tile([KAUG, T], bf16, tag=f"kt{h}", name=f"kt{h}")
                for h in range(HPC)]
        v_t = sb.tile([128, NQT, HPC, 65], bf16)
        attn_t = [sb.tile([128, T], bf16, tag=f"at{p}", name=f"at{p}")
                  for p in range(2)]
        xt_sb = sb.tile([128, KT_C, T], bf16)
        wq_sb = sb.tile([128, KT_C, CS], bf16)
        wk_sb = sb.tile([128, KT_C, CS], bf16)
        wv_sb = sb.tile([128, KT_C, CS], bf16)
        wo_sb = sb.tile([128, 2, C], bf16)

        augq_sb = sb.tile([128, T], bf16)
        augk_sb = sb.tile([128, T], bf16)

        # ---- front DMAs. DMA cost = dst per-partition line bytes, charged
        # serially on the issuing queue; SP and ACT HWDGE queues run in
        # parallel. First projection matmuls need wq[k0:4] + xt0[k0:4].
        xt_view = xT.rearrange("(k p) t -> p k t", p=128)
        wq_view = wqT.rearrange("(k p) c -> p k c", p=128)
        nc.sync.dma_start(out=wq_sb[:, 0:4, :], in_=wq_view[:, 0:4, :])
        nc.sync.dma_start(out=xt_sb[:, 0:4, 0:512], in_=xt_view[:, 0:4, 0:512])
        nc.sync.dma_start(out=wq_sb[:, 4:8, :], in_=wq_view[:, 4:8, :])
        nc.sync.dma_start(out=xt_sb[:, 4:8, 0:512], in_=xt_view[:, 4:8, 0:512])
        nc.sync.dma_start(out=wk_sb[:],
                          in_=wkT.rearrange("(k p) c -> p k c", p=128))
        nc.scalar.dma_start(out=augk_sb[:], in_=augk[:])
        nc.scalar.dma_start(out=augq_sb[:], in_=augq[:])
        nc.scalar.dma_start(out=wv_sb[:],
                            in_=wvT.rearrange("(k p) c -> p k c", p=128))
        nc.scalar.dma_start(out=xt_sb[:, :, 512:1024],
                            in_=xt_view[:, :, 512:1024])
        for tch in range(2, NQB):
            nc.sync.dma_start(out=xt_sb[:, :, tch * 512:(tch + 1) * 512],
                              in_=xt_view[:, :, tch * 512:(tch + 1) * 512])
        nc.sync.dma_start(
            out=wo_sb[:], in_=woT.rearrange("(k p) c -> p k c", p=128))
        nc.gpsimd.memset(v_t[:], 1.0)  # ones column preset (Pool is idle)
        # fan the aug rows out to their per-head tiles (4x-mode DVE copies)
        for h in range(HPC):
            nc.vector.tensor_copy(qt_t[h][64:68, :],
                                  augq_sb[32 * h:32 * h + 4, :])
            nc.vector.tensor_copy(kt_t[h][64:68, :],
                                  augk_sb[32 * h:32 * h + 4, :])



        cp_rr = [0]

        def copy_rr(dst, src):
            """Round-robin PSUM->SBUF copies between DVE and ACT (gpsimd
            cannot touch PSUM on real hardware)."""
            cp_rr[0] += 1
            if cp_rr[0] % 2 == 0:
                nc.vector.tensor_copy(dst, src)
            else:
                nc.scalar.copy(dst, src)

        def emit_proj_qk(tch, khalves=False):
            for (w_sb, dst) in ((wq_sb, qt_t), (wk_sb, kt_t)):
                # khalves: accumulate k 0:4 into both dt psums before k 4:8
                # so the PE stays busy while the second half-DMAs land
                korder = [(d, k) for kh in (0, 1) for d in (0, 1)
                          for k in range(4 * kh, 4 * kh + 4)] if khalves else \
                         [(d, k) for d in (0, 1) for k in range(KT_C)]
                ps = [psum.tile([128, 512], f32, tag="p1", bufs=2, name="qkps")
                      for _ in range(2)]
                for dt_i, k in korder:
                    nc.tensor.matmul(
                        ps[dt_i][:],
                        lhsT=w_sb[:, k, dt_i * 128:(dt_i + 1) * 128],
                        rhs=xt_sb[:, k, tch * 512:(tch + 1) * 512],
                        start=(k == 0), stop=(k == KT_C - 1),
                    )
                for dt_i in range(2):
                    for hl in range(2):
                        h = dt_i * 2 + hl
                        copy_rr(
                            dst[h][0:64, tch * 512:(tch + 1) * 512],
                            ps[dt_i][hl * 64:(hl + 1) * 64, :])

        def emit_proj_v(tch):
            for st in range(4 * tch, 4 * tch + 4):
                ps = psum.tile([128, 512], f32, tag="p1", bufs=2,
                               name=f"vps{st}")
                for k in range(KT_C):
                    nc.tensor.matmul(
                        ps[:, 0:CS],
                        lhsT=xt_sb[:, k, st * 128:(st + 1) * 128],
                        rhs=wv_sb[:, k, :],
                        start=(k == 0), stop=(k == KT_C - 1),
                    )
                nc.vector.tensor_copy(
                    v_t[:, st, :, 0:64],
                    ps[:, 0:CS].rearrange("p (h d) -> p h d", h=HPC))

        # attn_td tiles per (qb, pair): written by norm-muls, read by the
        # DMA transpose into attn_t
        td_tiles = {}

        def emit_att_st(qb, h):
            """ST matmuls + exp + causal mask for group (qb, h); the AV stage
            is deferred one group (psum accumulation groups are bank-scoped,
            so the four t-subtile accumulations must run jj-outer, which
            needs every pt of the group alive)."""
            kept = _kept(h, qb)
            pairs = [kept[i:i + 2] for i in range(0, len(kept), 2)]
            pt_of = {}
            for pr in pairs:
                sp = psum.tile([128, 2, 512], f32, tag="sp", bufs=2, name="sp")
                # both tiles compute the union window so one exp covers the
                # pair; extra columns are future-masked or below e^-TAU
                ulo = min(w[1] for w in pr)
                uhi = max(w[2] for w in pr)
                for j, (st, lo, hi) in enumerate(pr):
                    nc.tensor.matmul(
                        sp[:, j, ulo:uhi],
                        lhsT=kt_t[h][:, st * 128:(st + 1) * 128],
                        rhs=qt_t[h][:, qb * 512 + ulo:qb * 512 + uhi],
                        start=True, stop=True,
                    )
                pt = pt_pool.tile([128, 2, 512], bf16, tag="pt", name="ptt")
                npair = len(pr)
                nc.scalar.activation(pt[:, 0:npair, ulo:uhi],
                                     sp[:, 0:npair, ulo:uhi], EXP, scale=SCALE)
                for j, (st, lo, hi) in enumerate(pr):
                    rel = st - 4 * qb
                    if rel >= 0:  # causal mask on diagonal tiles
                        nc.gpsimd.affine_select(
                            pt[:, j, lo:hi], pt[:, j, lo:hi],
                            pattern=[[1, hi - lo]],
                            compare_op=mybir.AluOpType.is_ge,
                            fill=0.0, base=0,
                            channel_multiplier=-1,
                        )
                    pt_of[st] = (pt, j, hi)
            return (qb, h, kept, pt_of)

        def emit_att_av(ctx):
            """AV accumulation (jj-outer), normalize, transpose to (d,t)."""
            qb, h, kept, pt_of = ctx
            av_sts = [[st for (st, lo, hi) in kept
                       if hi >= (jj + 1) * 128 and lo <= jj * 128]
                      for jj in range(4)]
            av = psum.tile([128, 4, 65], f32, tag="av", bufs=2, name="av")
            for jj in range(4):
                sts = av_sts[jj]
                for st in sts:
                    pt, j, _ = pt_of[st]
                    nc.tensor.matmul(
                        av[:, jj, :],
                        lhsT=pt[:, j, jj * 128:(jj + 1) * 128],
                        rhs=v_t[:, st, h, :],
                        start=(st == sts[0]), stop=(st == sts[-1]),
                    )
            # normalize: per-partition (q position) reciprocal multiply
            p = h // 2
            key = (qb, p)
            if key not in td_tiles:
                td_tiles[key] = td_pool.tile([128, 4, 2, 64], bf16, tag="td",
                                             name=f"td{qb}_{p}")
            td = td_tiles[key]
            den = dn_pool.tile([128, 4], f32, tag="den", name="den")
            nc.vector.reciprocal(den[:], av[:, :, 64])
            for jj in range(4):
                nc.vector.tensor_scalar_mul(
                    td[:, jj, h % 2, :], av[:, jj, 0:64], den[:, jj:jj + 1])
            if h % 2 == 0:  # second head of the pair: transpose to (d, t)
                for jj in range(4):
                    nc.sync.dma_start_transpose(
                        out=attn_t[p][:, qb * 512 + jj * 128:
                                      qb * 512 + (jj + 1) * 128],
                        in_=td[:, jj, :, :])
                del td_tiles[key]

        def emit_outproj(qb, strips):
            for qt_i in [4 * qb + s for s in strips]:
                ob = ob_pool.tile([128, C], bf16, tag="ob", name="ob")
                for chn in range(2):
                    ps = psum.tile([128, 512], f32, tag="p1", bufs=2,
                                   name="ops")
                    for i, kt_i in enumerate((1, 0)):
                        nc.tensor.matmul(
                            ps[:],
                            lhsT=attn_t[kt_i][:, qt_i * 128:(qt_i + 1) * 128],
                            rhs=wo_sb[:, kt_i, chn * 512:(chn + 1) * 512],
                            start=(i == 0), stop=(i == 1),
                        )
                    copy_rr(ob[:, chn * 512:(chn + 1) * 512], ps[:])
                nc.sync.dma_start(
                    out=out[qt_i * 128:(qt_i + 1) * 128, :], in_=ob[:])

        # ---- software-pipelined emission: the AV stage of each attention
        # group is deferred two groups so PE never waits on that group's exps
        emit_proj_qk(0, khalves=True)
        emit_proj_v(0)
        pend = []

        def att(qb, h):
            pend.append(emit_att_st(qb, h))
            if len(pend) > 2:
                emit_att_av(pend.pop(0))

        for h in (3, 2, 1, 0):
            att(0, h)
        for qb in range(1, NQB):
            emit_proj_qk(qb)
            emit_proj_v(qb)
            att(qb, 3)
            att(qb, 2)
            emit_outproj(qb - 1, (0, 1))
            att(qb, 1)
            att(qb, 0)
            emit_outproj(qb - 1, (2, 3))
        for ctx in pend:
            emit_att_av(ctx)
        emit_outproj(NQB - 1, (0, 1, 2, 3))

    nc.finalize()
    return nc


def _bf16(a):
    import ml_dtypes
    return np.asarray(a, np.float32).astype(ml_dtypes.bfloat16)


def _host_inputs(x, Wq, Wk, Wv, Wo):
    import ml_dtypes
    bf = ml_dtypes.bfloat16
    slopes = _slopes(H)
    t_idx = np.arange(T, dtype=np.float64)
    in_maps = []
    for core in range(NCORES):
        b, g = core // 4, core % 4
        heads = [4 * j + g for j in range(HPC)]
        hs = np.concatenate([np.arange(h * D, (h + 1) * D) for h in heads])
        qa = np.zeros((128, T), dtype=bf)
        ka = np.zeros((128, T), dtype=bf)
        for j in range(HPC):
            sig = slopes[heads[j]]
            v = sig * t_idx / SCALE
            h1 = v.astype(bf)
            h2 = (v - h1.astype(np.float64)).astype(bf)
            h3 = (v - h1.astype(np.float64) - h2.astype(np.float64)).astype(bf)
            qa[32 * j] = (-v).astype(bf)
            qa[32 * j + 1:32 * j + 4] = bf(1.0)
            ka[32 * j] = bf(1.0)
            ka[32 * j + 1] = h1
            ka[32 * j + 2] = h2
            ka[32 * j + 3] = h3
        in_maps.append({
            "xT": _bf16(np.ascontiguousarray(x[b].T)),
            "wqT": _bf16(np.ascontiguousarray(Wq[hs, :].T)),
            "wkT": _bf16(np.ascontiguousarray(Wk[hs, :].T)),
            "wvT": _bf16(np.ascontiguousarray(Wv[hs, :].T)),
            "woT": _bf16(np.ascontiguousarray(Wo[:, hs].T)),
            "augq": qa,
            "augk": ka,
        })
    return in_maps


def get_program():
    global _PROGRAM
    if _PROGRAM is None:
        _PROGRAM = _build_program()
    return _PROGRAM


def kernel(x, Wq, Wk, Wv, Wo, _trace=False):
    from concourse.bass_utils import run_bass_kernel_spmd

    x = np.asarray(x, dtype=np.float32)
    nc = get_program()
    in_maps = _host_inputs(x, np.asarray(Wq, np.float32),
                           np.asarray(Wk, np.float32),
                           np.asarray(Wv, np.float32),
                           np.asarray(Wo, np.float32))
    res = run_bass_kernel_spmd(nc, in_maps, list(range(NCORES)), trace=_trace)
    kernel.last_results = res
    outs = [np.asarray(res.results[i]["out"], dtype=np.float32)
            for i in range(NCORES)]
    full = np.empty((B, T, C), dtype=np.float32)
    for b in range(B):
        full[b] = outs[4 * b] + outs[4 * b + 1] + outs[4 * b + 2] + outs[4 * b + 3]
    return full


# revision 26
# speedup vs baseline: 1.0872x; 1.0139x over previous
"""ALiBi causal attention (B=2, T=2048, C=1024, H=16, D=64, fp32) on 8 trn2 cores.

Sharding: core i -> batch b = i//4, group g = i%4; slot j of core g holds head
4j+g (sorted slope grouping: slot j covers heads {4j..4j+3}, so the compiled
skip pattern for slot j only needs to cover slope(4j+3), the smallest in the
slot, and every core does identical work).

All matmuls in bf16 (1 PE cycle/row at any output width):
  phase 1: xT (C,T) bf16 in SBUF; QT/KT in (d,t) layout with 4 aug rows:
           QTe rows 64..67 = [-slope*t/scale, 1, 1, 1]
           KTe rows 64..67 = [1, h1, h2, h3] where h1+h2+h3 = slope*s/scale
           split across three bf16 rows (24 mantissa bits -> exact through the
           f32 PSUM accumulate; the t-term's bf16 error is constant per t and
           cancels in softmax). V in (t,d) layout + ones column (denominator).
  phase 2: per (head, 512-wide q-block): kept s-tiles from the ALiBi band
           (tau=18) with per-tile column windows; ST pair -> one ACT exp ->
           PT bf16; causal mask via gpsimd affine_select; AV *flipped*:
           avT[t,65] += PT_subtile^T @ V  (65-wide bf16 matmuls, and diagonal
           s-tiles only feed their causal t-subtiles). Normalize per-partition
           (reciprocal + tensor_scalar_mul, no broadcast matmul), then
           DMA-transpose [128,128] tiles into the (d,t) attn layout.
  phase 3: out_partial = attnT^T @ WoT per 128-row strip -> bf16 DMA out.

Emission is software-pipelined (proj chunk k+1 / out-proj strips of qb-1
interleave with attention of qb) and all input DMAs are bf16, split across
the SP and ACT HWDGE queues. Host sums the 4 bf16 partials per batch.
"""

import math
import sys

import numpy as np

for _p in ("/opt/trn_rl_repo", "/root/.axon_site/_ro/trn_rl_repo"):
    try:
        import concourse  # noqa: F401
        break
    except ImportError:
        if _p not in sys.path:
            sys.path.insert(0, _p)

B, T, C, H, D = 2, 2048, 1024, 16, 64
HPC = 4          # heads per core (one per slot)
CS = HPC * D     # 256 channels per core
SCALE = D ** -0.5
NCORES = 8
KAUG = 68        # 64 head dims + t-term row + 3-way s-term split
TAU = 15.0       # drop softmax terms with slope*gap > TAU (< 7e-4 rel mass)

NQT = T // 128   # 16 s/t tiles
NQB = T // 512   # 4 q blocks
KT_C = C // 128  # 8 contraction chunks for projections


def _slopes(n_heads: int) -> np.ndarray:
    i = np.arange(1, n_heads + 1, dtype=np.float64)
    return np.power(2.0, -8.0 * i / n_heads)


# worst (smallest) slope in slot j is head 4j+3
_SLOT_SIG = [float(_slopes(H)[4 * j + 3]) for j in range(HPC)]
_TSIG = [TAU / s for s in _SLOT_SIG]


def _window(j: int, qb: int, st: int):
    """Column window [lo, hi) of s-tile st within q-block qb for slot j,
    or None if the whole tile is below the ALiBi band."""
    rel = st - 4 * qb
    if rel > 3:
        return None
    hi = 128 * rel + 128 + _TSIG[j]
    hi = min(512, 128 * math.ceil(hi / 128))
    lo = max(0, 128 * rel)
    if hi <= lo:
        return None
    return lo, int(hi)


def _kept(j: int, qb: int):
    out = []
    for st in range(4 * qb + 4):
        w = _window(j, qb, st)
        if w is not None:
            out.append((st, w[0], w[1]))
    return out


_PROGRAM = None


def _build_program():
    from contextlib import ExitStack

    import concourse.tile as tile
    from concourse import bacc, mybir

    f32 = mybir.dt.float32
    bf16 = mybir.dt.bfloat16
    EXP = mybir.ActivationFunctionType.Exp

    nc = bacc.Bacc("TRN2", target_bir_lowering=False, debug=False,
                   num_devices=NCORES)
    xT = nc.declare_dram_parameter("xT", [C, T], bf16, isOutput=False)
    wqT = nc.declare_dram_parameter("wqT", [C, CS], bf16, isOutput=False)
    wkT = nc.declare_dram_parameter("wkT", [C, CS], bf16, isOutput=False)
    wvT = nc.declare_dram_parameter("wvT", [C, CS], bf16, isOutput=False)
    woT = nc.declare_dram_parameter("woT", [CS, C], bf16, isOutput=False)
    # aug rows padded to [128, T] with head h's 4 rows at partition 32h:
    # DMA cost is per-partition line bytes (partition count is free), and
    # the on-device fan-out copies need start partitions in {0,32,64,96}
    augq = nc.declare_dram_parameter("augq", [128, T], bf16, isOutput=False)
    augk = nc.declare_dram_parameter("augk", [128, T], bf16, isOutput=False)
    out = nc.declare_dram_parameter("out", [T, C], bf16, isOutput=True)

    with nc.allow_low_precision(reason="bf16 compute, f32 PSUM accumulate"), \
         tile.TileContext(nc) as tc, ExitStack() as ctx:
        sb = ctx.enter_context(tc.tile_pool(name="sb", bufs=1))
        psum = ctx.enter_context(tc.tile_pool(name="psum", bufs=1, space="PSUM"))
        pt_pool = ctx.enter_context(tc.tile_pool(name="pt", bufs=24))
        td_pool = ctx.enter_context(tc.tile_pool(name="td", bufs=4))
        dn_pool = ctx.enter_context(tc.tile_pool(name="dn", bufs=4))
        ob_pool = ctx.enter_context(tc.tile_pool(name="ob", bufs=2))

        qt_t = [sb.tile([KAUG, T], bf16, tag=f"qt{h}", name=f"qt{h}")
                for h in range(HPC)]
        kt_t = [sb.tile([KAUG, T], bf16, tag=f"kt{h}", name=f"kt{h}")
                for h in range(HPC)]
        v_t = sb.tile([128, NQT, HPC, 65], bf16)
        attn_t = [sb.tile([128, T], bf16, tag=f"at{p}", name=f"at{p}")
                  for p in range(2)]
        xt_sb = sb.tile([128, KT_C, T], bf16)
        wq_sb = sb.tile([128, KT_C, CS], bf16)
        wk_sb = sb.tile([128, KT_C, CS], bf16)
        wv_sb = sb.tile([128, KT_C, CS], bf16)
        wo_sb = sb.tile([128, 2, C], bf16)

        augq_sb = sb.tile([128, T], bf16)
        augk_sb = sb.tile([128, T], bf16)

        # ---- front DMAs. DMA cost = dst per-partition line bytes, charged
        # serially on the issuing queue; SP and ACT HWDGE queues run in
        # parallel. First projection matmuls need wq[k0:4] + xt0[k0:4].
        xt_view = xT.rearrange("(k p) t -> p k t", p=128)
        wq_view = wqT.rearrange("(k p) c -> p k c", p=128)
        wk_view = wkT.rearrange("(k p) c -> p k c", p=128)
        nc.sync.dma_start(out=wq_sb[:, 0:4, :], in_=wq_view[:, 0:4, :])
        nc.sync.dma_start(out=xt_sb[:, 0:4, 0:512], in_=xt_view[:, 0:4, 0:512])
        nc.sync.dma_start(out=wq_sb[:, 4:8, :], in_=wq_view[:, 4:8, :])
        nc.sync.dma_start(out=wk_sb[:, 0:4, :], in_=wk_view[:, 0:4, :])
        nc.sync.dma_start(out=xt_sb[:, 4:8, 0:512], in_=xt_view[:, 4:8, 0:512])
        nc.sync.dma_start(out=wk_sb[:, 4:8, :], in_=wk_view[:, 4:8, :])
        nc.scalar.dma_start(out=augk_sb[:], in_=augk[:])
        nc.scalar.dma_start(out=augq_sb[:], in_=augq[:])
        nc.scalar.dma_start(out=wv_sb[:],
                            in_=wvT.rearrange("(k p) c -> p k c", p=128))
        for tch in range(1, NQB):
            nc.sync.dma_start(out=xt_sb[:, :, tch * 512:(tch + 1) * 512],
                              in_=xt_view[:, :, tch * 512:(tch + 1) * 512])
        nc.sync.dma_start(
            out=wo_sb[:], in_=woT.rearrange("(k p) c -> p k c", p=128))
        nc.gpsimd.memset(v_t[:], 1.0)  # ones column preset (Pool is idle)
        # fan the aug rows out to their per-head tiles (4x-mode DVE copies)
        for h in range(HPC):
            nc.vector.tensor_copy(qt_t[h][64:68, :],
                                  augq_sb[32 * h:32 * h + 4, :])
            nc.vector.tensor_copy(kt_t[h][64:68, :],
                                  augk_sb[32 * h:32 * h + 4, :])



        cp_rr = [0]

        def copy_rr(dst, src):
            """Round-robin PSUM->SBUF copies between DVE and ACT (gpsimd
            cannot touch PSUM on real hardware)."""
            cp_rr[0] += 1
            if cp_rr[0] % 2 == 0:
                nc.vector.tensor_copy(dst, src)
            else:
                nc.scalar.copy(dst, src)

        def emit_proj_qk(tch, khalves=False):
            for (w_sb, dst) in ((wq_sb, qt_t), (wk_sb, kt_t)):
                # khalves: accumulate k 0:4 into both dt psums before k 4:8
                # so the PE stays busy while the second half-DMAs land
                korder = [(d, k) for kh in (0, 1) for d in (0, 1)
                          for k in range(4 * kh, 4 * kh + 4)] if khalves else \
                         [(d, k) for d in (0, 1) for k in range(KT_C)]
                ps = [psum.tile([128, 512], f32, tag="p1", bufs=2, name="qkps")
                      for _ in range(2)]
                for dt_i, k in korder:
                    nc.tensor.matmul(
                        ps[dt_i][:],
                        lhsT=w_sb[:, k, dt_i * 128:(dt_i + 1) * 128],
                        rhs=xt_sb[:, k, tch * 512:(tch + 1) * 512],
                        start=(k == 0), stop=(k == KT_C - 1),
                    )
                for dt_i in range(2):
                    for hl in range(2):
                        h = dt_i * 2 + hl
                        nc.vector.tensor_copy(
                            dst[h][0:64, tch * 512:(tch + 1) * 512],
                            ps[dt_i][hl * 64:(hl + 1) * 64, :])

        def emit_proj_v(tch):
            for st in range(4 * tch, 4 * tch + 4):
                ps = psum.tile([128, 512], f32, tag="p1", bufs=2,
                               name=f"vps{st}")
                for k in range(KT_C):
                    nc.tensor.matmul(
                        ps[:, 0:CS],
                        lhsT=xt_sb[:, k, st * 128:(st + 1) * 128],
                        rhs=wv_sb[:, k, :],
                        start=(k == 0), stop=(k == KT_C - 1),
                    )
                nc.vector.tensor_copy(
                    v_t[:, st, :, 0:64],
                    ps[:, 0:CS].rearrange("p (h d) -> p h d", h=HPC))

        # attn_td tiles per (qb, pair): written by norm-muls, read by the
        # DMA transpose into attn_t
        td_tiles = {}

        def emit_att_st(qb, h):
            """ST matmuls + exp + causal mask for group (qb, h); the AV stage
            is deferred one group (psum accumulation groups are bank-scoped,
            so the four t-subtile accumulations must run jj-outer, which
            needs every pt of the group alive)."""
            kept = _kept(h, qb)
            pairs = [kept[i:i + 2] for i in range(0, len(kept), 2)]
            pt_of = {}
            for pr in pairs:
                sp = psum.tile([128, 2, 512], f32, tag="sp", bufs=2, name="sp")
                # both tiles compute the union window so one exp covers the
                # pair; extra columns are future-masked or below e^-TAU
                ulo = min(w[1] for w in pr)
                uhi = max(w[2] for w in pr)
                for j, (st, lo, hi) in enumerate(pr):
                    nc.tensor.matmul(
                        sp[:, j, ulo:uhi],
                        lhsT=kt_t[h][:, st * 128:(st + 1) * 128],
                        rhs=qt_t[h][:, qb * 512 + ulo:qb * 512 + uhi],
                        start=True, stop=True,
                    )
                pt = pt_pool.tile([128, 2, 512], bf16, tag="pt", name="ptt")
                npair = len(pr)
                nc.scalar.activation(pt[:, 0:npair, ulo:uhi],
                                     sp[:, 0:npair, ulo:uhi], EXP, scale=SCALE)
                for j, (st, lo, hi) in enumerate(pr):
                    rel = st - 4 * qb
                    if rel >= 0:  # causal mask on diagonal tiles
                        nc.gpsimd.affine_select(
                            pt[:, j, lo:hi], pt[:, j, lo:hi],
                            pattern=[[1, hi - lo]],
                            compare_op=mybir.AluOpType.is_ge,
                            fill=0.0, base=0,
                            channel_multiplier=-1,
                        )
                    pt_of[st] = (pt, j, hi)
            return (qb, h, kept, pt_of)

        def emit_att_av(ctx):
            """AV accumulation (jj-outer), normalize, transpose to (d,t)."""
            qb, h, kept, pt_of = ctx
            av_sts = [[st for (st, lo, hi) in kept
                       if hi >= (jj + 1) * 128 and lo <= jj * 128]
                      for jj in range(4)]
            av = psum.tile([128, 4, 65], f32, tag="av", bufs=2, name="av")
            for jj in range(4):
                sts = av_sts[jj]
                for st in sts:
                    pt, j, _ = pt_of[st]
                    nc.tensor.matmul(
                        av[:, jj, :],
                        lhsT=pt[:, j, jj * 128:(jj + 1) * 128],
                        rhs=v_t[:, st, h, :],
                        start=(st == sts[0]), stop=(st == sts[-1]),
                    )
            # normalize: per-partition (q position) reciprocal multiply
            p = h // 2
            key = (qb, p)
            if key not in td_tiles:
                td_tiles[key] = td_pool.tile([128, 4, 2, 64], bf16, tag="td",
                                             name=f"td{qb}_{p}")
            td = td_tiles[key]
            den = dn_pool.tile([128, 4], f32, tag="den", name="den")
            nc.vector.reciprocal(den[:], av[:, :, 64])
            for jj in range(4):
                nc.vector.tensor_scalar_mul(
                    td[:, jj, h % 2, :], av[:, jj, 0:64], den[:, jj:jj + 1])
            if h % 2 == 0:  # second head of the pair: transpose to (d, t)
                for jj in range(4):
                    nc.sync.dma_start_transpose(
                        out=attn_t[p][:, qb * 512 + jj * 128:
                                      qb * 512 + (jj + 1) * 128],
                        in_=td[:, jj, :, :])
                del td_tiles[key]

        def emit_outproj(qb, strips):
            for qt_i in [4 * qb + s for s in strips]:
                ob = ob_pool.tile([128, C], bf16, tag="ob", name="ob")
                for chn in range(2):
                    ps = psum.tile([128, 512], f32, tag="p1", bufs=2,
                                   name="ops")
                    for i, kt_i in enumerate((1, 0)):
                        nc.tensor.matmul(
                            ps[:],
                            lhsT=attn_t[kt_i][:, qt_i * 128:(qt_i + 1) * 128],
                            rhs=wo_sb[:, kt_i, chn * 512:(chn + 1) * 512],
                            start=(i == 0), stop=(i == 1),
                        )
                    copy_rr(ob[:, chn * 512:(chn + 1) * 512], ps[:])
                nc.sync.dma_start(
                    out=out[qt_i * 128:(qt_i + 1) * 128, :], in_=ob[:])

        # ---- software-pipelined emission: the AV stage of each attention
        # group is deferred two groups so PE never waits on that group's exps
        emit_proj_qk(0, khalves=True)
        emit_proj_v(0)
        pend = []

        def att(qb, h):
            pend.append(emit_att_st(qb, h))
            if len(pend) > 2:
                emit_att_av(pend.pop(0))

        for h in (3, 2, 1, 0):
            att(0, h)
        for qb in range(1, NQB):
            emit_proj_qk(qb)
            emit_proj_v(qb)
            att(qb, 3)
            att(qb, 2)
            emit_outproj(qb - 1, (0, 1))
            att(qb, 1)
            att(qb, 0)
            emit_outproj(qb - 1, (2, 3))
        for ctx in pend:
            emit_att_av(ctx)
        emit_outproj(NQB - 1, (0, 1, 2, 3))

    nc.finalize()
    return nc


def _bf16(a):
    import ml_dtypes
    return np.asarray(a, np.float32).astype(ml_dtypes.bfloat16)


def _host_inputs(x, Wq, Wk, Wv, Wo):
    import ml_dtypes
    bf = ml_dtypes.bfloat16
    slopes = _slopes(H)
    t_idx = np.arange(T, dtype=np.float64)
    in_maps = []
    for core in range(NCORES):
        b, g = core // 4, core % 4
        heads = [4 * j + g for j in range(HPC)]
        hs = np.concatenate([np.arange(h * D, (h + 1) * D) for h in heads])
        qa = np.zeros((128, T), dtype=bf)
        ka = np.zeros((128, T), dtype=bf)
        for j in range(HPC):
            sig = slopes[heads[j]]
            v = sig * t_idx / SCALE
            h1 = v.astype(bf)
            h2 = (v - h1.astype(np.float64)).astype(bf)
            h3 = (v - h1.astype(np.float64) - h2.astype(np.float64)).astype(bf)
            qa[32 * j] = (-v).astype(bf)
            qa[32 * j + 1:32 * j + 4] = bf(1.0)
            ka[32 * j] = bf(1.0)
            ka[32 * j + 1] = h1
            ka[32 * j + 2] = h2
            ka[32 * j + 3] = h3
        in_maps.append({
            "xT": _bf16(np.ascontiguousarray(x[b].T)),
            "wqT": _bf16(np.ascontiguousarray(Wq[hs, :].T)),
            "wkT": _bf16(np.ascontiguousarray(Wk[hs, :].T)),
            "wvT": _bf16(np.ascontiguousarray(Wv[hs, :].T)),
            "woT": _bf16(np.ascontiguousarray(Wo[:, hs].T)),
            "augq": qa,
            "augk": ka,
        })
    return in_maps


def get_program():
    global _PROGRAM
    if _PROGRAM is None:
        _PROGRAM = _build_program()
    return _PROGRAM


def kernel(x, Wq, Wk, Wv, Wo, _trace=False):
    from concourse.bass_utils import run_bass_kernel_spmd

    x = np.asarray(x, dtype=np.float32)
    nc = get_program()
    in_maps = _host_inputs(x, np.asarray(Wq, np.float32),
                           np.asarray(Wk, np.float32),
                           np.asarray(Wv, np.float32),
                           np.asarray(Wo, np.float32))
    res = run_bass_kernel_spmd(nc, in_maps, list(range(NCORES)), trace=_trace)
    kernel.last_results = res
    outs = [np.asarray(res.results[i]["out"], dtype=np.float32)
            for i in range(NCORES)]
    full = np.empty((B, T, C), dtype=np.float32)
    for b in range(B):
        full[b] = outs[4 * b] + outs[4 * b + 1] + outs[4 * b + 2] + outs[4 * b + 3]
    return full


# revision 31
# speedup vs baseline: 1.1775x; 1.0831x over previous
"""ALiBi causal attention (B=2, T=2048, C=1024, H=16, D=64, fp32) on 8 trn2 cores.

Sharding: core i -> batch b = i//4, group g = i%4; slot j of core g holds head
4j+g (sorted slope grouping: slot j covers heads {4j..4j+3}, so the compiled
skip pattern for slot j only needs to cover slope(4j+3), the smallest in the
slot, and every core does identical work).

All matmuls in bf16 (1 PE cycle/row at any output width):
  phase 1: xT (C,T) bf16 in SBUF; QT/KT in (d,t) layout with 4 aug rows:
           QTe rows 64..67 = [-slope*t/scale, 1, 1, 1]
           KTe rows 64..67 = [1, h1, h2, h3] where h1+h2+h3 = slope*s/scale
           split across three bf16 rows (24 mantissa bits -> exact through the
           f32 PSUM accumulate; the t-term's bf16 error is constant per t and
           cancels in softmax). V in (t,d) layout + ones column (denominator).
  phase 2: per (head, 512-wide q-block): kept s-tiles from the ALiBi band
           (tau=18) with per-tile column windows; ST pair -> one ACT exp ->
           PT bf16; causal mask via gpsimd affine_select; AV *flipped*:
           avT[t,65] += PT_subtile^T @ V  (65-wide bf16 matmuls, and diagonal
           s-tiles only feed their causal t-subtiles). Normalize per-partition
           (reciprocal + tensor_scalar_mul, no broadcast matmul), then
           DMA-transpose [128,128] tiles into the (d,t) attn layout.
  phase 3: out_partial = attnT^T @ WoT per 128-row strip -> bf16 DMA out.

Emission is software-pipelined (proj chunk k+1 / out-proj strips of qb-1
interleave with attention of qb) and all input DMAs are bf16, split across
the SP and ACT HWDGE queues. Host sums the 4 bf16 partials per batch.
"""

import math
import sys

import numpy as np

for _p in ("/opt/trn_rl_repo", "/root/.axon_site/_ro/trn_rl_repo"):
    try:
        import concourse  # noqa: F401
        break
    except ImportError:
        if _p not in sys.path:
            sys.path.insert(0, _p)

B, T, C, H, D = 2, 2048, 1024, 16, 64
HPC = 4          # heads per core (one per slot)
CS = HPC * D     # 256 channels per core
SCALE = D ** -0.5
NCORES = 8
KAUG = 68        # 64 head dims + t-term row + 3-way s-term split
TAU = 15.0       # drop softmax terms with slope*gap > TAU (< 7e-4 rel mass)

NQT = T // 128   # 16 s/t tiles
NQB = T // 512   # 4 q blocks
KT_C = C // 128  # 8 contraction chunks for projections


def _slopes(n_heads: int) -> np.ndarray:
    i = np.arange(1, n_heads + 1, dtype=np.float64)
    return np.power(2.0, -8.0 * i / n_heads)


# worst (smallest) slope in slot j is head 4j+3
_SLOT_SIG = [float(_slopes(H)[4 * j + 3]) for j in range(HPC)]
_TSIG = [TAU / s for s in _SLOT_SIG]


def _window(j: int, qb: int, st: int):
    """Column window [lo, hi) of s-tile st within q-block qb for slot j,
    or None if the whole tile is below the ALiBi band."""
    rel = st - 4 * qb
    if rel > 3:
        return None
    hi = 128 * rel + 128 + _TSIG[j]
    hi = min(512, 128 * math.ceil(hi / 128))
    lo = max(0, 128 * rel)
    if hi <= lo:
        return None
    return lo, int(hi)


def _kept(j: int, qb: int):
    out = []
    for st in range(4 * qb + 4):
        w = _window(j, qb, st)
        if w is not None:
            out.append((st, w[0], w[1]))
    return out


_PROGRAM = None


def _build_program():
    from contextlib import ExitStack

    import concourse.tile as tile
    from concourse import bacc, mybir

    f32 = mybir.dt.float32
    bf16 = mybir.dt.bfloat16
    EXP = mybir.ActivationFunctionType.Exp

    nc = bacc.Bacc("TRN2", target_bir_lowering=False, debug=False,
                   num_devices=NCORES)
    xT = nc.declare_dram_parameter("xT", [C, T], bf16, isOutput=False)
    wqT = nc.declare_dram_parameter("wqT", [C, CS], bf16, isOutput=False)
    wkT = nc.declare_dram_parameter("wkT", [C, CS], bf16, isOutput=False)
    wvT = nc.declare_dram_parameter("wvT", [C, CS], bf16, isOutput=False)
    woT = nc.declare_dram_parameter("woT", [CS, C], bf16, isOutput=False)
    # aug rows padded to [128, T] with head h's 4 rows at partition 32h:
    # DMA cost is per-partition line bytes (partition count is free), and
    # the on-device fan-out copies need start partitions in {0,32,64,96}
    augq = nc.declare_dram_parameter("augq", [128, T], bf16, isOutput=False)
    augk = nc.declare_dram_parameter("augk", [128, T], bf16, isOutput=False)
    out = nc.declare_dram_parameter("out", [T, C], bf16, isOutput=True)

    with nc.allow_low_precision(reason="bf16 compute, f32 PSUM accumulate"), \
         tile.TileContext(nc) as tc, ExitStack() as ctx:
        sb = ctx.enter_context(tc.tile_pool(name="sb", bufs=1))
        psum = ctx.enter_context(tc.tile_pool(name="psum", bufs=1, space="PSUM"))
        pt_pool = ctx.enter_context(tc.tile_pool(name="pt", bufs=32))
        td_pool = ctx.enter_context(tc.tile_pool(name="td", bufs=4))
        dn_pool = ctx.enter_context(tc.tile_pool(name="dn", bufs=4))
        ob_pool = ctx.enter_context(tc.tile_pool(name="ob", bufs=2))

        qt_t = [sb.tile([KAUG, T], bf16, tag=f"qt{h}", name=f"qt{h}")
                for h in range(HPC)]
        kt_t = [sb.tile([KAUG, T], bf16, tag=f"kt{h}", name=f"kt{h}")
                for h in range(HPC)]
        v_t = sb.tile([128, NQT, HPC, 65], bf16)
        attn_t = [sb.tile([128, T], bf16, tag=f"at{p}", name=f"at{p}")
                  for p in range(2)]
        xt_sb = sb.tile([128, KT_C, T], bf16)
        wq_sb = sb.tile([128, KT_C, CS], bf16)
        wk_sb = sb.tile([128, KT_C, CS], bf16)
        wv_sb = sb.tile([128, KT_C, CS], bf16)
        wo_sb = sb.tile([128, 2, C], bf16)

        augq_sb = sb.tile([128, T], bf16)
        augk_sb = sb.tile([128, T], bf16)

        # ---- front DMAs. DMA cost = dst per-partition line bytes, charged
        # serially on the issuing queue; SP and ACT HWDGE queues run in
        # parallel. First projection matmuls need wq[k0:4] + xt0[k0:4].
        xt_view = xT.rearrange("(k p) t -> p k t", p=128)
        wq_view = wqT.rearrange("(k p) c -> p k c", p=128)
        wk_view = wkT.rearrange("(k p) c -> p k c", p=128)
        nc.sync.dma_start(out=wq_sb[:, 0:4, :], in_=wq_view[:, 0:4, :])
        nc.sync.dma_start(out=xt_sb[:, 0:4, 0:512], in_=xt_view[:, 0:4, 0:512])
        nc.sync.dma_start(out=wq_sb[:, 4:8, :], in_=wq_view[:, 4:8, :])
        nc.sync.dma_start(out=wk_sb[:, 0:4, :], in_=wk_view[:, 0:4, :])
        nc.sync.dma_start(out=xt_sb[:, 4:8, 0:512], in_=xt_view[:, 4:8, 0:512])
        nc.sync.dma_start(out=wk_sb[:, 4:8, :], in_=wk_view[:, 4:8, :])
        nc.scalar.dma_start(out=augk_sb[:], in_=augk[:])
        nc.scalar.dma_start(out=augq_sb[:], in_=augq[:])
        nc.scalar.dma_start(out=wv_sb[:],
                            in_=wvT.rearrange("(k p) c -> p k c", p=128))
        for tch in range(1, NQB):
            nc.sync.dma_start(out=xt_sb[:, :, tch * 512:(tch + 1) * 512],
                              in_=xt_view[:, :, tch * 512:(tch + 1) * 512])
        nc.sync.dma_start(
            out=wo_sb[:], in_=woT.rearrange("(k p) c -> p k c", p=128))
        nc.gpsimd.memset(v_t[:], 1.0)  # ones column preset (Pool is idle)



        def emit_proj_qk(tch, khalves=False):
            for (w_sb, dst) in ((wq_sb, qt_t), (wk_sb, kt_t)):
                # khalves: accumulate k 0:4 into both dt psums before k 4:8
                # so the PE stays busy while the second half-DMAs land
                korder = [(d, k) for kh in (0, 1) for d in (0, 1)
                          for k in range(4 * kh, 4 * kh + 4)] if khalves else \
                         [(d, k) for d in (0, 1) for k in range(KT_C)]
                ps = [psum.tile([128, 512], f32, tag="p1", bufs=2, name="qkps")
                      for _ in range(2)]
                for dt_i, k in korder:
                    nc.tensor.matmul(
                        ps[dt_i][:],
                        lhsT=w_sb[:, k, dt_i * 128:(dt_i + 1) * 128],
                        rhs=xt_sb[:, k, tch * 512:(tch + 1) * 512],
                        start=(k == 0), stop=(k == KT_C - 1),
                    )
                for dt_i in range(2):
                    for hl in range(2):
                        h = dt_i * 2 + hl
                        nc.vector.tensor_copy(
                            dst[h][0:64, tch * 512:(tch + 1) * 512],
                            ps[dt_i][hl * 64:(hl + 1) * 64, :])

        def emit_proj_v(tch):
            for st in range(4 * tch, 4 * tch + 4):
                ps = psum.tile([128, 512], f32, tag="p1", bufs=2,
                               name=f"vps{st}")
                for k in range(KT_C):
                    nc.tensor.matmul(
                        ps[:, 0:CS],
                        lhsT=xt_sb[:, k, st * 128:(st + 1) * 128],
                        rhs=wv_sb[:, k, :],
                        start=(k == 0), stop=(k == KT_C - 1),
                    )
                nc.vector.tensor_copy(
                    v_t[:, st, :, 0:64],
                    ps[:, 0:CS].rearrange("p (h d) -> p h d", h=HPC))

        # attn_td tiles per (qb, pair): written by norm-muls, read by the
        # DMA transpose into attn_t
        td_tiles = {}

        def emit_att_st(qb, h):
            """ST matmuls + exp + causal mask for group (qb, h); the AV stage
            is deferred one group (psum accumulation groups are bank-scoped,
            so the four t-subtile accumulations must run jj-outer, which
            needs every pt of the group alive)."""
            kept = _kept(h, qb)
            pairs = [kept[i:i + 2] for i in range(0, len(kept), 2)]
            pt_of = {}
            for pr in pairs:
                sp = psum.tile([128, 2, 512], f32, tag="sp", bufs=2, name="sp")
                # both tiles compute the union window so one exp covers the
                # pair; extra columns are future-masked or below e^-TAU
                ulo = min(w[1] for w in pr)
                uhi = max(w[2] for w in pr)
                for j, (st, lo, hi) in enumerate(pr):
                    nc.tensor.matmul(
                        sp[:, j, ulo:uhi],
                        lhsT=kt_t[h][:, st * 128:(st + 1) * 128],
                        rhs=qt_t[h][:, qb * 512 + ulo:qb * 512 + uhi],
                        start=True, stop=True,
                    )
                pt = pt_pool.tile([128, 2, 512], bf16, tag="pt", name="ptt")
                npair = len(pr)
                nc.scalar.activation(pt[:, 0:npair, ulo:uhi],
                                     sp[:, 0:npair, ulo:uhi], EXP, scale=SCALE)
                for j, (st, lo, hi) in enumerate(pr):
                    rel = st - 4 * qb
                    if rel >= 0:  # causal mask on diagonal tiles
                        nc.gpsimd.affine_select(
                            pt[:, j, lo:hi], pt[:, j, lo:hi],
                            pattern=[[1, hi - lo]],
                            compare_op=mybir.AluOpType.is_ge,
                            fill=0.0, base=0,
                            channel_multiplier=-1,
                        )
                    pt_of[st] = (pt, j, hi)
            return (qb, h, kept, pt_of)

        def emit_att_av(ctx):
            """AV accumulation (jj-outer), normalize, transpose to (d,t)."""
            qb, h, kept, pt_of = ctx
            av_sts = [[st for (st, lo, hi) in kept
                       if hi >= (jj + 1) * 128 and lo <= jj * 128]
                      for jj in range(4)]
            av = psum.tile([128, 4, 65], f32, tag="av", bufs=2, name="av")
            for jj in range(4):
                sts = av_sts[jj]
                for st in sts:
                    pt, j, _ = pt_of[st]
                    nc.tensor.matmul(
                        av[:, jj, :],
                        lhsT=pt[:, j, jj * 128:(jj + 1) * 128],
                        rhs=v_t[:, st, h, :],
                        start=(st == sts[0]), stop=(st == sts[-1]),
                    )
            # normalize: per-partition (q position) reciprocal multiply
            p = h // 2
            key = (qb, p)
            if key not in td_tiles:
                td_tiles[key] = td_pool.tile([128, 4, 2, 64], bf16, tag="td",
                                             name=f"td{qb}_{p}")
            td = td_tiles[key]
            den = dn_pool.tile([128, 4], f32, tag="den", name="den")
            nc.vector.reciprocal(den[:], av[:, :, 64])
            for jj in range(4):
                nc.vector.tensor_scalar_mul(
                    td[:, jj, h % 2, :], av[:, jj, 0:64], den[:, jj:jj + 1])
            if h % 2 == 0:  # second head of the pair: transpose to (d, t)
                for jj in range(4):
                    nc.sync.dma_start_transpose(
                        out=attn_t[p][:, qb * 512 + jj * 128:
                                      qb * 512 + (jj + 1) * 128],
                        in_=td[:, jj, :, :])
                del td_tiles[key]

        def emit_outproj(qb, strips):
            for qt_i in [4 * qb + s for s in strips]:
                ob = ob_pool.tile([128, C], bf16, tag="ob", name="ob")
                for chn in range(2):
                    ps = psum.tile([128, 512], f32, tag="p1", bufs=2,
                                   name="ops")
                    for i, kt_i in enumerate((1, 0)):
                        nc.tensor.matmul(
                            ps[:],
                            lhsT=attn_t[kt_i][:, qt_i * 128:(qt_i + 1) * 128],
                            rhs=wo_sb[:, kt_i, chn * 512:(chn + 1) * 512],
                            start=(i == 0), stop=(i == 1),
                        )
                    nc.vector.tensor_copy(ob[:, chn * 512:(chn + 1) * 512],
                                          ps[:])
                nc.sync.dma_start(
                    out=out[qt_i * 128:(qt_i + 1) * 128, :], in_=ob[:])

        # ---- software-pipelined emission: the AV stage of each attention
        # group is deferred two groups so PE never waits on that group's exps
        emit_proj_qk(0, khalves=True)
        # fan the aug rows out to their per-head tiles (4x-mode DVE copies);
        # emitted after proj0 so they don't delay the first psum copies
        for h in range(HPC):
            nc.vector.tensor_copy(qt_t[h][64:68, :],
                                  augq_sb[32 * h:32 * h + 4, :])
            nc.vector.tensor_copy(kt_t[h][64:68, :],
                                  augk_sb[32 * h:32 * h + 4, :])
        emit_proj_v(0)
        pend = []

        def att(qb, h):
            pend.append(emit_att_st(qb, h))
            if len(pend) > 3:
                emit_att_av(pend.pop(0))

        for h in (3, 2, 1, 0):
            att(0, h)
        for qb in range(1, NQB):
            emit_proj_qk(qb)
            emit_proj_v(qb)
            att(qb, 3)
            att(qb, 2)
            emit_outproj(qb - 1, (0, 1))
            att(qb, 1)
            att(qb, 0)
            emit_outproj(qb - 1, (2, 3))
        for ctx in pend:
            emit_att_av(ctx)
        emit_outproj(NQB - 1, (0, 1, 2, 3))

    nc.finalize()
    return nc


def _bf16(a):
    import ml_dtypes
    return np.asarray(a, np.float32).astype(ml_dtypes.bfloat16)


def _host_inputs(x, Wq, Wk, Wv, Wo):
    import ml_dtypes
    bf = ml_dtypes.bfloat16
    slopes = _slopes(H)
    t_idx = np.arange(T, dtype=np.float64)
    in_maps = []
    for core in range(NCORES):
        b, g = core // 4, core % 4
        heads = [4 * j + g for j in range(HPC)]
        hs = np.concatenate([np.arange(h * D, (h + 1) * D) for h in heads])
        qa = np.zeros((128, T), dtype=bf)
        ka = np.zeros((128, T), dtype=bf)
        for j in range(HPC):
            sig = slopes[heads[j]]
            v = sig * t_idx / SCALE
            h1 = v.astype(bf)
            h2 = (v - h1.astype(np.float64)).astype(bf)
            h3 = (v - h1.astype(np.float64) - h2.astype(np.float64)).astype(bf)
            qa[32 * j] = (-v).astype(bf)
            qa[32 * j + 1:32 * j + 4] = bf(1.0)
            ka[32 * j] = bf(1.0)
            ka[32 * j + 1] = h1
            ka[32 * j + 2] = h2
            ka[32 * j + 3] = h3
        in_maps.append({
            "xT": _bf16(np.ascontiguousarray(x[b].T)),
            "wqT": _bf16(np.ascontiguousarray(Wq[hs, :].T)),
            "wkT": _bf16(np.ascontiguousarray(Wk[hs, :].T)),
            "wvT": _bf16(np.ascontiguousarray(Wv[hs, :].T)),
            "woT": _bf16(np.ascontiguousarray(Wo[:, hs].T)),
            "augq": qa,
            "augk": ka,
        })
    return in_maps


def get_program():
    global _PROGRAM
    if _PROGRAM is None:
        _PROGRAM = _build_program()
    return _PROGRAM


def kernel(x, Wq, Wk, Wv, Wo, _trace=False):
    from concourse.bass_utils import run_bass_kernel_spmd

    x = np.asarray(x, dtype=np.float32)
    nc = get_program()
    in_maps = _host_inputs(x, np.asarray(Wq, np.float32),
                           np.asarray(Wk, np.float32),
                           np.asarray(Wv, np.float32),
                           np.asarray(Wo, np.float32))
    res = run_bass_kernel_spmd(nc, in_maps, list(range(NCORES)), trace=_trace)
    kernel.last_results = res
    outs = [np.asarray(res.results[i]["out"], dtype=np.float32)
            for i in range(NCORES)]
    full = np.empty((B, T, C), dtype=np.float32)
    for b in range(B):
        full[b] = outs[4 * b] + outs[4 * b + 1] + outs[4 * b + 2] + outs[4 * b + 3]
    return full
